# revision 4
# baseline (speedup 1.0000x reference)
"""DANet3D dual-attention kernel for Trainium2 (8 NeuronCores, Bass/Tile).

Sharding: x -> proj p [2, 64, 8000]; 8 cores = 2 batches x 4 query-blocks
of 2000 positions.  Each core receives the full batch projection (keys /
values / channel attention) plus its own query block and computes its
[64, 2000] slice of the output.

Position attention (per batch):
  E[n,m] = (Wq p_n + bq).(Wk p_m + bk)
         = p_n^T M p_m + w.p_m + row-constants,   M = Wq^T Wk, w = Wk^T bq
  softmax_m is invariant to row-constants, and exp(w.p_m) = g[m] is a
  per-key scale commuting with the softmax numerator/denominator:
  attn = rowscale( exp(p_n^T M p_m) * g[m] ).
  On device: kp = M p (one projection, no q/k), then a flash loop over 63
  key tiles:  F = exp(kp_tile^T p_q)  [128 keys x 1000 queries per exp op],
  U[c,n] += vt[m,c] F[m,n]  where vt cols 0:64 = gamma_p*g[m]*(Wv p + bv),
  col 64 = g[m] (ones-trick -> U[64] is the softmax denominator).

Channel attention is tiny and replicated: ac2 = gamma_c*attn_c^T + 2I is
folded into one matmul  outc2 = ac2^T p_q = gamma_c*out_c + 2x.
Final: out = U[0:64] * (1/U[64]) + outc2.

Dtypes: the n^2 flash matmuls (QK, AV) and their operands are bf16
(full-rate PE streaming + FWL weight loads; PSUM accumulation stays fp32).
Exactness-critical matmuls (outc2 -> the 2x term, the 1/S broadcast) are
plain fp32 so the gamma=0 case is exact.
"""

from contextlib import ExitStack

import ml_dtypes
import numpy as np

import concourse.bass as bass
import concourse.mybir as mybir
import concourse.tile as tile
from concourse import bacc
from concourse.bass import ds, ts
from concourse.bass_utils import run_bass_kernel_spmd
from concourse.masks import make_identity

F32 = mybir.dt.float32
BF16 = mybir.dt.bfloat16
AF = mybir.ActivationFunctionType
ALU = mybir.AluOpType
AX = mybir.AxisListType

B, C, D, H, W = 2, 64, 20, 20, 20
N = D * H * W            # 8000
MT = 128                 # key (m) tile size
NMT = 63                 # number of m tiles
NPAD = MT * NMT          # 8064, zero-padded key range
NQ = N // 4              # 2000 queries per core
NH = NQ // 2             # 1000 = one exp-op worth of queries
KCH = 504                # kp projection chunk (16 x 504 = 8064)
CH = (512, 488)          # query sub-chunks, each within one PSUM bank
GRP = (0, 16, 32, 48, NMT)  # g-scale groups (pipeline vt against main loop)
NCORES = 8


def build_danet(ctx, tc, io):
    nc = tc.nc
    xbb, xq, xqb, mpT, wvx = io["xbb"], io["xq"], io["xqb"], io["mpT"], io["wvx"]
    gc, gp, eye2, out_d = io["gc"], io["gp"], io["eye2"], io["out"]

    persist = ctx.enter_context(tc.tile_pool(name="persist", bufs=1))
    fs_pool = ctx.enter_context(tc.tile_pool(name="fs", bufs=3))

    pab = persist.tile([65, NPAD], BF16)     # bf16 proj + ones row (host)
    paq = persist.tile([64, NQ], F32)        # query block fp32 (outc2)
    paqb = persist.tile([64, NQ], BF16)      # query block bf16 (QK rhs)
    kp = persist.tile([64, NPAD], BF16)      # M @ p
    vraw = persist.tile([128, NMT, 65], F32) # unscaled [vT+bv | w.p]
    vt = persist.tile([128, NMT, 65], BF16)  # scaled [vT*g*gp | g]
    pt = persist.tile([128, NMT, 64], BF16)  # projT tiles (channel attn)
    gexp = persist.tile([128, NMT], F32)     # g = exp(w.p)
    gg = persist.tile([128, NMT], F32)       # gamma_p * g
    mpT_s = persist.tile([64, 64], BF16)
    wvx_s = persist.tile([65, 129], BF16)
    gc_s = persist.tile([64, 1], F32)
    gp_s = persist.tile([128, 1], F32)
    eye2_s = persist.tile([64, 64], F32)
    id64 = persist.tile([64, 64], F32)
    ones_s = persist.tile([1, 64], F32)
    ee = persist.tile([64, 64], F32)
    ac2 = persist.tile([64, 64], F32)
    mx = persist.tile([64, 1], F32)
    sc = persist.tile([64, 1], F32)
    rc = persist.tile([64, 1], F32)
    rcg = persist.tile([64, 1], F32)
    oc_sb = persist.tile([64, NQ], F32)      # gamma_c*out_c + 2x
    rec = persist.tile([1, NQ], F32)
    bc_sb = persist.tile([64, NQ], F32)
    out_sb = persist.tile([64, NQ], F32)

    # ---- load inputs ----
    nc.sync.dma_start(out=mpT_s, in_=mpT)
    nc.sync.dma_start(out=wvx_s, in_=wvx)
    nc.sync.dma_start(out=gc_s, in_=gc)
    nc.sync.dma_start(out=gp_s, in_=gp)
    nc.sync.dma_start(out=eye2_s, in_=eye2)
    nc.sync.dma_start(out=paq, in_=xq)
    nc.sync.dma_start(out=paqb, in_=xqb)
    NXCH = 8
    xw = NPAD // NXCH
    for i in range(NXCH):
        nc.sync.dma_start(out=pab[:, ts(i, xw)], in_=xbb[:, ts(i, xw)])
    make_identity(nc, id64)
    nc.vector.memset(ones_s, 1.0)

    # ---- prologue: projections + channel attention ----
    with tc.tile_pool(name="ps_pro", bufs=2, space="PSUM") as pro, \
         tc.tile_pool(name="ps_pro1", bufs=1, space="PSUM") as pro1, \
         tc.tile_pool(name="ps_oc", bufs=2, space="PSUM") as ocp:

        # kp = M @ p  (bf16 in, fp32 psum)
        for i in range(NPAD // KCH):
            kp_ps = pro.tile([64, KCH], F32)
            nc.tensor.matmul(kp_ps, mpT_s, pab[0:64, ts(i, KCH)],
                             start=True, stop=True)
            nc.vector.tensor_copy(out=kp[:, ts(i, KCH)], in_=kp_ps)

        # vt tiles [128, 129] = pa_tile^T @ [WvT+bv | w | I]
        for t in range(NMT):
            vt_ps = pro.tile([128, 129], F32)
            nc.tensor.matmul(vt_ps, pab[:, ts(t, MT)], wvx_s,
                             start=True, stop=True)
            nc.vector.tensor_copy(out=vraw[:, t, :], in_=vt_ps[:, 0:65])
            nc.vector.tensor_copy(out=pt[:, t, :], in_=vt_ps[:, 65:129])

        # per-group: g = exp(w.p), gg = gamma_p*g, scale value cols
        for gi in range(len(GRP) - 1):
            lo, hi = GRP[gi], GRP[gi + 1]
            nc.scalar.activation(
                out=gexp[:, lo:hi].rearrange("p (t o) -> p t o", o=1),
                in_=vraw[:, lo:hi, 64:65], func=AF.Exp)
            nc.vector.tensor_scalar_mul(out=gg[:, lo:hi],
                                        in0=gexp[:, lo:hi], scalar1=gp_s)
            for t in range(lo, hi):
                nc.vector.tensor_scalar_mul(out=vt[:, t, 0:64],
                                            in0=vraw[:, t, 0:64],
                                            scalar1=gg[:, t : t + 1])
                nc.vector.tensor_copy(out=vt[:, t, 64:65],
                                      in_=gexp[:, t : t + 1])
        nc.vector.memset(vt[64:128, NMT - 1, :], 0.0)  # pad keys m>=8000

        # channel attention on projT tiles
        ec_ps = pro1.tile([64, 64], F32)
        for t in range(NMT):
            ptile = pt[:, t, :]
            nc.tensor.matmul(ec_ps, ptile, ptile,
                             start=(t == 0), stop=(t == NMT - 1))
        nc.vector.tensor_reduce(out=mx, in_=ec_ps, axis=AX.X, op=ALU.max,
                                negate=True)
        nc.scalar.activation(out=ee, in_=ec_ps, func=AF.Exp, bias=mx)
        nc.vector.tensor_reduce(out=sc, in_=ee, axis=AX.X, op=ALU.add)
        nc.vector.reciprocal(out=rc, in_=sc)
        nc.vector.tensor_mul(out=rcg, in0=rc, in1=gc_s)
        nc.vector.tensor_scalar_mul(out=ee, in0=ee, scalar1=rcg)
        at_ps = pro1.tile([64, 64], F32)
        nc.tensor.transpose(at_ps, ee, id64)
        nc.vector.tensor_add(out=ac2, in0=at_ps, in1=eye2_s)

        # outc2 = ac2^T @ p_q = gamma_c*out_c + 2x  (fp32: exact 2x)
        for j in range(4):
            oc_ps = ocp.tile([64, 500], F32)
            nc.tensor.matmul(oc_ps, ac2, paq[:, ts(j, 500)],
                             start=True, stop=True)
            nc.vector.tensor_copy(out=oc_sb[:, ts(j, 500)], in_=oc_ps)

    # ---- main flash loop ----
    with tc.tile_pool(name="ps_u", bufs=1, space="PSUM") as up:
        u_ps = [up.tile([65, 1024], F32, name=f"u{h}", tag=f"u{h}")
                for h in range(2)]
        with tc.tile_pool(name="ps_f", bufs=2, space="PSUM") as fp:
            for t in range(NMT):
                kpt = kp[:, ts(t, MT)]
                vtt = vt[:, t, :]
                for h in range(2):
                    f_ps = fp.tile([128, 1024], F32)
                    for off, w_ in zip((0, 512), CH):
                        nc.tensor.matmul(f_ps[:, ds(off, w_)], kpt,
                                         paqb[:, ds(h * NH + off, w_)],
                                         start=True, stop=True)
                    f_sb = fs_pool.tile([128, 1000], BF16)
                    nc.scalar.activation(out=f_sb, in_=f_ps[:, 0:1000],
                                         func=AF.Exp)
                    for off, w_ in zip((0, 512), CH):
                        nc.tensor.matmul(u_ps[h][:, ds(off, w_)], vtt,
                                         f_sb[:, ds(off, w_)],
                                         start=(t == 0), stop=(t == NMT - 1))

        # ---- epilogue: normalize + combine ----
        with tc.tile_pool(name="ps_bc", bufs=2, space="PSUM") as bcp:
            for h in range(2):
                uh = u_ps[h]
                nc.vector.reciprocal(out=rec[:, ds(h * NH, NH)],
                                     in_=uh[64:65, 0:1000])
                bc_ps = bcp.tile([64, 1024], F32)
                for off, w_ in zip((0, 512), CH):
                    nc.tensor.matmul(bc_ps[:, ds(off, w_)], ones_s,
                                     rec[:, ds(h * NH + off, w_)],
                                     start=True, stop=True)
                nc.vector.tensor_copy(out=bc_sb[:, ds(h * NH, NH)],
                                      in_=bc_ps[:, 0:1000])
                o_h = out_sb[:, ds(h * NH, NH)]
                nc.vector.tensor_mul(out=o_h, in0=uh[0:64, 0:1000],
                                     in1=bc_sb[:, ds(h * NH, NH)])
                nc.vector.tensor_add(out=o_h, in0=o_h,
                                     in1=oc_sb[:, ds(h * NH, NH)])
            nc.sync.dma_start(out=out_d, in_=out_sb)


def _mk_io(nc):
    io = {}
    io["xbb"] = nc.dram_tensor("xbb", [65, NPAD], BF16,
                               kind="ExternalInput").ap()
    io["xq"] = nc.dram_tensor("xq", [64, NQ], F32, kind="ExternalInput").ap()
    io["xqb"] = nc.dram_tensor("xqb", [64, NQ], BF16,
                               kind="ExternalInput").ap()
    io["mpT"] = nc.dram_tensor("mpT", [64, 64], BF16,
                               kind="ExternalInput").ap()
    io["wvx"] = nc.dram_tensor("wvx", [65, 129], BF16,
                               kind="ExternalInput").ap()
    io["gc"] = nc.dram_tensor("gc", [64, 1], F32, kind="ExternalInput").ap()
    io["gp"] = nc.dram_tensor("gp", [128, 1], F32, kind="ExternalInput").ap()
    io["eye2"] = nc.dram_tensor("eye2", [64, 64], F32,
                                kind="ExternalInput").ap()
    io["out"] = nc.dram_tensor("out", [64, NQ], F32,
                               kind="ExternalOutput").ap()
    return io


_CACHE = {}


def build_program():
    if "nc" not in _CACHE:
        nc = bacc.Bacc("TRN2", target_bir_lowering=False, debug=False,
                       num_devices=NCORES)
        io = _mk_io(nc)
        with tile.TileContext(nc) as tc, ExitStack() as ctx:
            build_danet(ctx, tc, io)
        nc.compile()
        _CACHE["nc"] = nc
    return _CACHE["nc"]


def make_in_maps(x, Wq, bq, Wk, bk, Wv, bv, gamma_c, gamma_p):
    f = np.float32
    bf = ml_dtypes.bfloat16
    proj = np.asarray(x, f).reshape(B, C, N)
    Wq, bq, Wk, bk = (np.asarray(a, f) for a in (Wq, bq, Wk, bk))
    Wv, bv = np.asarray(Wv, f), np.asarray(bv, f)
    gamma_c = float(np.asarray(gamma_c).reshape(-1)[0])
    gamma_p = float(np.asarray(gamma_p).reshape(-1)[0])

    mpT = (Wq.T @ Wk).T.astype(bf)       # lhsT for kp = M @ p
    w = (Wk.T @ bq).astype(f)            # per-key bias -> column scale
    wvx = np.zeros((65, 129), f)
    wvx[0:64, 0:64] = Wv.T
    wvx[64, 0:64] = bv
    wvx[0:64, 64] = w
    wvx[0:64, 65:129] = np.eye(64, dtype=f)
    wvx = wvx.astype(bf)
    gc = np.full((64, 1), gamma_c, f)
    gp = np.full((128, 1), gamma_p, f)
    eye2 = (2.0 * np.eye(64)).astype(f)

    in_maps = []
    for core in range(NCORES):
        b, qb = divmod(core, 4)
        xbuf = np.zeros((65, NPAD), f)
        xbuf[0:64, 0:N] = proj[b]
        xbuf[64, :] = 1.0
        xqf = np.ascontiguousarray(proj[b][:, qb * NQ:(qb + 1) * NQ])
        in_maps.append({"xbb": xbuf.astype(bf), "xq": xqf,
                        "xqb": xqf.astype(bf), "mpT": mpT, "wvx": wvx,
                        "gc": gc, "gp": gp, "eye2": eye2})
    return in_maps


def run_on_cores(in_maps, **kw):
    nc = build_program()
    return run_bass_kernel_spmd(nc, in_maps, core_ids=list(range(NCORES)),
                                **kw)


def kernel(**inputs):
    x = np.asarray(inputs["x"])
    in_maps = make_in_maps(
        inputs["x"], inputs["Wq"], inputs["bq"], inputs["Wk"], inputs["bk"],
        inputs["Wv"], inputs["bv"], inputs["gamma_c"], inputs["gamma_p"])
    res = run_on_cores(in_maps)
    out = np.zeros((B, C, N), np.float32)
    for core in range(NCORES):
        b, qb = divmod(core, 4)
        out[b][:, qb * NQ:(qb + 1) * NQ] = res.results[core]["out"]
    return out.reshape(x.shape).astype(x.dtype, copy=False)
